# revision 6
# baseline (speedup 1.0000x reference)
"""nn_MultiHeadAttention fused single-core Trainium2 kernel.

The axon tunnel to the devices runs at ~60-85 MB/s and is serialized across
cores, so wall-clock is dominated by host<->device bytes plus per-instruction
sync overheads, not FLOPs. The attention therefore runs fused on TWO
NeuronCores (one per batch element, weights replicated) with bf16 I/O
(~47MB total wire) instead of replicating 37MB of activations to 8 cores.

Shaw relative-position terms are handled with a flat-buffer skew trick:
P = Q @ pe_k^T is written row-major to DRAM and re-read with an overlapping
access pattern (partition stride 128, element stride 1) which materializes
P[q, k-q+64] as a plain rectangular tile; out-of-band cells are masked and
the two clipped tails are applied as per-row activation biases during exp.
The same trick extracts the banded weight sums for the pe_v output term.
"""

import numpy as np
import ml_dtypes

B, S, HID, NH, HD = 2, 1500, 1024, 16, 64
SP = 1536  # k padded to 12*128 for full-block XBAR transposes
NQT = 12
QTS = [128] * 11 + [92]
BF = ml_dtypes.bfloat16

_CACHE = {}


def _build(level=99):
    if level == 1:
        level = 14
    elif level == 2:
        level = 22
    elif level >= 3:
        level = 33
    import concourse.bacc as bacc
    import concourse.mybir as mybir
    from concourse.tile import TileContext
    from concourse import masks as cmasks

    F32, BF16 = mybir.dt.float32, mybir.dt.bfloat16
    AF = mybir.ActivationFunctionType
    ALU = mybir.AluOpType

    nc = bacc.Bacc("TRN2", target_bir_lowering=False, debug=False, num_devices=2)
    qT = nc.declare_dram_parameter("qT", [HID, S], BF16, isOutput=False)
    kT = nc.declare_dram_parameter("kT", [HID, S], BF16, isOutput=False)
    vT = nc.declare_dram_parameter("vT", [HID, S], BF16, isOutput=False)
    wq = nc.declare_dram_parameter("wq", [NH, HID, HD], BF16, isOutput=False)
    wk = nc.declare_dram_parameter("wk", [NH, HID, HD], BF16, isOutput=False)
    wv = nc.declare_dram_parameter("wv", [NH, HID, HD], BF16, isOutput=False)
    bqT = nc.declare_dram_parameter("bqT", [HD, NH], F32, isOutput=False)
    bkT = nc.declare_dram_parameter("bkT", [HD, NH], F32, isOutput=False)
    bvT = nc.declare_dram_parameter("bvT", [HD, NH], F32, isOutput=False)
    pkT = nc.declare_dram_parameter("pkT", [HD, 129], BF16, isOutput=False)
    pvi = nc.declare_dram_parameter("pvi", [127, HD], BF16, isOutput=False)
    pvt = nc.declare_dram_parameter("pvt", [2, HD], BF16, isOutput=False)
    wfc = nc.declare_dram_parameter("wfc", [HID, HID], BF16, isOutput=False)
    bfr = nc.declare_dram_parameter("bfr", [1, HID], BF16, isOutput=False)
    out = nc.declare_dram_parameter("o", [S, HID], BF16, isOutput=True)

    pbufs = [nc.dram_tensor(f"pbuf{i}", [128 * 129], BF16) for i in range(2)]
    ebufs = [nc.dram_tensor(f"ebuf{i}", [64 + 128 * 256], BF16) for i in range(2)]

    with TileContext(nc) as tc:
        with (
            tc.tile_pool(name="cst", bufs=1) as cst,
            tc.tile_pool(name="big", bufs=1) as big,
            tc.tile_pool(name="hd", bufs=2) as hdp,
            tc.tile_pool(name="sm", bufs=5) as sm,
            tc.tile_pool(name="psS", bufs=1, space="PSUM") as psSp,
            tc.tile_pool(name="psO", bufs=2, space="PSUM") as psOp,
            tc.tile_pool(name="psA", bufs=3, space="PSUM") as psAp,
        ):
            # ---- constants ----
            ident = cst.tile([128, 128], BF16, tag="ident")
            cmasks.make_identity(nc, ident[:])
            ones = cst.tile([1, 128], BF16, tag="ones")
            nc.vector.memset(ones[:], 1.0)
            # band masks on [128, 256]: d = c - p - 64
            mB = cst.tile([128, 256], BF16, tag="mB")
            mL = cst.tile([128, 256], BF16, tag="mL")
            mR = cst.tile([128, 256], BF16, tag="mR")
            mL2 = cst.tile([128, 256], BF16, tag="mL2")
            mR2 = cst.tile([128, 256], BF16, tag="mR2")
            nc.vector.memset(mB[:], 1.0)
            # keep where d >= -64  <=>  c - p >= 0
            nc.gpsimd.affine_select(out=mB[:], in_=mB[:], compare_op=ALU.is_ge,
                                    fill=0.0, base=0, channel_multiplier=-1,
                                    pattern=[[1, 256]])
            # keep where d <= 64  <=>  128 + p - c >= 0
            nc.gpsimd.affine_select(out=mB[:], in_=mB[:], compare_op=ALU.is_ge,
                                    fill=0.0, base=128, channel_multiplier=1,
                                    pattern=[[-1, 256]])
            nc.vector.memset(mL[:], 1.0)
            # keep where d < -64  <=>  p - c - 1 >= 0
            nc.gpsimd.affine_select(out=mL[:], in_=mL[:], compare_op=ALU.is_ge,
                                    fill=0.0, base=-1, channel_multiplier=1,
                                    pattern=[[-1, 256]])
            nc.vector.memset(mR[:], 1.0)
            # keep where d > 64  <=>  c - p - 129 >= 0
            nc.gpsimd.affine_select(out=mR[:], in_=mR[:], compare_op=ALU.is_ge,
                                    fill=0.0, base=-129, channel_multiplier=-1,
                                    pattern=[[1, 256]])
            nc.vector.memset(mL2[:], 1.0)
            # keep where d <= -64  <=>  p - c >= 0
            nc.gpsimd.affine_select(out=mL2[:], in_=mL2[:], compare_op=ALU.is_ge,
                                    fill=0.0, base=0, channel_multiplier=1,
                                    pattern=[[-1, 256]])
            nc.vector.memset(mR2[:], 1.0)
            # keep where d >= 64  <=>  c - p - 128 >= 0
            nc.gpsimd.affine_select(out=mR2[:], in_=mR2[:], compare_op=ALU.is_ge,
                                    fill=0.0, base=-128, channel_multiplier=-1,
                                    pattern=[[1, 256]])

            pkT_sb = cst.tile([HD, 129], BF16, tag="pkT")
            nc.sync.dma_start(pkT_sb[:], pkT[:, :])
            pvi_sb = cst.tile([127, HD], BF16, tag="pvi")
            nc.sync.dma_start(pvi_sb[:], pvi[:, :])
            pvt_sb = cst.tile([2, HD], BF16, tag="pvt")
            nc.sync.dma_start(pvt_sb[:], pvt[:, :])
            bqT_sb = cst.tile([HD, NH], F32, tag="bqT")
            nc.sync.dma_start(bqT_sb[:], bqT[:, :])
            bkT_sb = cst.tile([HD, NH], F32, tag="bkT")
            nc.sync.dma_start(bkT_sb[:], bkT[:, :])
            bvT_sb = cst.tile([HD, NH], F32, tag="bvT")
            nc.sync.dma_start(bvT_sb[:], bvT[:, :])
            bfr_sb = cst.tile([1, HID], BF16, tag="bfr")
            nc.sync.dma_start(bfr_sb[:], bfr[:, :])
            wfc_sb = cst.tile([128, 8, HID], BF16, tag="wfc")
            nc.sync.dma_start(wfc_sb[:], wfc.rearrange("(c p) o -> p c o", p=128))

            scnt = 0
            for b in range(1):
                xq = big.tile([128, 8, S], BF16, tag="xq")
                nc.sync.dma_start(xq[:], qT.rearrange("(c p) s -> p c s", p=128))
                xk = big.tile([128, 8, S], BF16, tag="xk")
                nc.sync.dma_start(xk[:], kT.rearrange("(c p) s -> p c s", p=128))
                xv = big.tile([128, 8, S], BF16, tag="xv")
                nc.sync.dma_start(xv[:], vT.rearrange("(c p) s -> p c s", p=128))
                hid_sb = big.tile([128, 8, S], BF16, tag="hid")
                if level < 14:
                    nc.vector.memset(hid_sb[:], 0.0)

                for n in range(NH):
                    wqh = hdp.tile([128, 8, HD], BF16, tag="wqh")
                    nc.sync.dma_start(wqh[:], wq[n].rearrange("(c p) d -> p c d", p=128))
                    wkh = hdp.tile([128, 8, HD], BF16, tag="wkh")
                    nc.sync.dma_start(wkh[:], wk[n].rearrange("(c p) d -> p c d", p=128))
                    wvh = hdp.tile([128, 8, HD], BF16, tag="wvh")
                    nc.sync.dma_start(wvh[:], wv[n].rearrange("(c p) d -> p c d", p=128))

                    # head projections: K^T, Q^T, V^T  [64, SP]
                    kTn = hdp.tile([HD, SP], BF16, tag="kTn")
                    qTn = hdp.tile([HD, SP], BF16, tag="qTn")
                    vTn = hdp.tile([HD, SP], BF16, tag="vTn", bufs=2)
                    for j0, w, xsrc, wsrc, bsrc, dst in (
                        [(j0, w, xk, wkh, bkT_sb, kTn) for j0, w in ((0, 512), (512, 512), (1024, 476))]
                        + [(j0, w, xq, wqh, bqT_sb, qTn) for j0, w in ((0, 512), (512, 512), (1024, 476))]
                        + [(j0, w, xv, wvh, bvT_sb, vTn) for j0, w in ((0, 512), (512, 512), (1024, 476))]
                    ):
                        psK = psAp.tile([128, 512], F32, tag="psA")
                        for c in range(8):
                            nc.tensor.matmul(psK[:HD, :w], wsrc[:, c, :],
                                             xsrc[:, c, j0:j0 + w],
                                             start=(c == 0), stop=(c == 7))
                        nc.scalar.add(dst[:, j0:j0 + w], psK[:HD, :w], bsrc[:, n:n + 1])
                    nc.vector.memset(vTn[:, S:SP], 0.0)
                    v_sb = hdp.tile([128, NQT, HD], BF16, tag="v_sb")
                    nc.sync.dma_start_transpose(v_sb[:, :, :], vTn[:, 0:SP])

                    for qt in range(NQT if level >= 11 else 0):
                        tw = QTS[qt]
                        q0 = qt * 128
                        kst = max(0, q0 - 64)
                        ken = min(S, q0 + tw + 64)
                        wB = ken - kst
                        cA = kst - (q0 - 64)

                        # P = Q @ pe_k^T for this q-tile, to DRAM flat
                        if level >= 22:
                            psP = psAp.tile([128, 512], F32, tag="psA")
                            nc.tensor.matmul(psP[:tw, 0:129], qTn[:, q0:q0 + tw],
                                             pkT_sb[:], start=True, stop=True)
                            plpr = sm.tile([128, 2], F32, tag="plpr")
                            nc.vector.tensor_copy(plpr[:tw, :], psP[:tw, 0:129:128])
                            pf = sm.tile([128, 129], BF16, tag="pf")
                            nc.scalar.copy(pf[:tw, :], psP[:tw, 0:129])
                            pb = pbufs[scnt % 2]
                            nc.sync.dma_start(
                                pb[0:tw * 129].rearrange("(r j) -> r j", j=129), pf[:tw, :])

                        # content scores into PSUM
                        psS = psSp.tile([128, SP], F32, tag="psS")
                        for j0 in ((0, 512, 1024) if level >= 11.5 else ()):
                            nc.tensor.matmul(psS[:tw, j0:j0 + 512],
                                             qTn[:, q0:q0 + tw], kTn[:, j0:j0 + 512],
                                             start=True, stop=True,
                                             skip_group_check=True)

                        # banded rel-key bias: skew-read P, mask, add into psS
                        if level >= 22:
                            band = sm.tile([128, 256], BF16, tag="band")
                            src = pb[:]
                            v = src.ap
                            v.clear()
                            v.extend([[128, tw], [1, wB]])
                            src.offset = cA
                            nc.sync.dma_start(band[:tw, 0:wB], src)
                            g = sm.tile([128, 256], BF16, tag="g")
                            nc.vector.tensor_tensor(out=g[:tw, 0:wB], in0=band[:tw, 0:wB],
                                                    in1=mB[:tw, cA:cA + wB], op=ALU.mult)
                            u1 = sm.tile([128, 256], BF16, tag="u1")
                            nc.vector.scalar_tensor_tensor(
                                out=u1[:tw, 0:wB], in0=mL[:tw, cA:cA + wB],
                                scalar=plpr[:tw, 0:1], in1=g[:tw, 0:wB],
                                op0=ALU.mult, op1=ALU.add)
                            u2 = sm.tile([128, 256], BF16, tag="u2")
                            nc.vector.scalar_tensor_tensor(
                                out=u2[:tw, 0:wB], in0=mR[:tw, cA:cA + wB],
                                scalar=plpr[:tw, 1:2], in1=u1[:tw, 0:wB],
                                op0=ALU.mult, op1=ALU.add)
                            # accumulate into psS via identity matmul, split at bank edges
                            for e0, e1 in ((kst, min(ken, 512)), (max(kst, 512), min(ken, 1024)), (max(kst, 1024), ken)):
                                if e1 > e0:
                                    nc.tensor.matmul(
                                        psS[:tw, e0:e1], ident[:tw, :tw],
                                        u2[:tw, e0 - kst:e1 - kst],
                                        start=False, stop=True,
                                        skip_group_check=True)

                        # exp with per-region tail biases; accumulate row sums
                        E = hdp.tile([128, SP], BF16, tag="E")
                        zL = sm.tile([128, 1], F32, tag="zL")
                        zB = sm.tile([128, 1], F32, tag="zB")
                        zR = sm.tile([128, 1], F32, tag="zR")
                        if level >= 12:
                            if level >= 22 and kst > 0:
                                nc.scalar.activation(E[:tw, 0:kst], psS[:tw, 0:kst],
                                                     AF.Exp, bias=plpr[:tw, 0:1],
                                                     accum_out=zL[:tw, :])
                            else:
                                nc.vector.memset(zL[:tw, :], 0.0)
                            if level >= 22:
                                nc.scalar.activation(E[:tw, kst:ken], psS[:tw, kst:ken],
                                                     AF.Exp, accum_out=zB[:tw, :])
                            else:
                                nc.scalar.activation(E[:tw, 0:S], psS[:tw, 0:S],
                                                     AF.Exp, accum_out=zB[:tw, :])
                            if level >= 22 and ken < S:
                                nc.scalar.activation(E[:tw, ken:S], psS[:tw, ken:S],
                                                     AF.Exp, bias=plpr[:tw, 1:2],
                                                     accum_out=zR[:tw, :])
                            else:
                                nc.vector.memset(zR[:tw, :], 0.0)
                            nc.vector.memset(E[:, S:SP], 0.0)
                            zz = sm.tile([128, 1], F32, tag="zz")
                            nc.vector.tensor_add(zz[:tw, :], zL[:tw, :], zB[:tw, :])
                            nc.vector.tensor_add(zz[:tw, :], zz[:tw, :], zR[:tw, :])
                            rz = sm.tile([128, 1], F32, tag="rz")
                            nc.vector.reciprocal(rz[:tw, :], zz[:tw, :])
                            nc.vector.tensor_scalar_mul(E[:tw, 0:S], E[:tw, 0:S],
                                                        rz[:tw, 0:1])

                        # clipped-tail weight sums for the pe_v term
                        if level >= 33:
                            cLt = sm.tile([128, 1], F32, tag="cLt")
                            cRt = sm.tile([128, 1], F32, tag="cRt")
                            scr = sm.tile([128, 256], BF16, tag="scr")
                            nc.vector.scalar_tensor_tensor(
                                out=scr[:tw, 0:wB], in0=E[:tw, kst:ken], scalar=1.0,
                                in1=mL2[:tw, cA:cA + wB], op0=ALU.mult, op1=ALU.mult,
                                accum_out=cLt[:tw, :])
                            scr2 = sm.tile([128, 256], BF16, tag="scr2")
                            nc.vector.scalar_tensor_tensor(
                                out=scr2[:tw, 0:wB], in0=E[:tw, kst:ken], scalar=1.0,
                                in1=mR2[:tw, cA:cA + wB], op0=ALU.mult, op1=ALU.mult,
                                accum_out=cRt[:tw, :])
                            LR = sm.tile([128, 2], BF16, tag="LR")
                            nc.vector.scalar_tensor_tensor(
                                out=LR[:tw, 0:1], in0=zL[:tw, :], scalar=rz[:tw, 0:1],
                                in1=cLt[:tw, :], op0=ALU.mult, op1=ALU.add)
                            nc.vector.scalar_tensor_tensor(
                                out=LR[:tw, 1:2], in0=zR[:tw, :], scalar=rz[:tw, 0:1],
                                in1=cRt[:tw, :], op0=ALU.mult, op1=ALU.add)
                            psLR = psAp.tile([128, 1024], BF16, tag="psA")
                            nc.tensor.transpose(psLR[:2, :tw], LR[:tw, :], ident[:tw, :tw])
                            lrT = sm.tile([2, 128], BF16, tag="lrT")
                            nc.scalar.copy(lrT[:, :tw], psLR[:2, :tw])

                            # banded weights: E band to DRAM, skew-read diagonals
                            eb = ebufs[scnt % 2]
                            nc.sync.dma_start(
                                eb[64:64 + tw * 256].rearrange("(r c) -> r c", c=256)[:, 0:wB],
                                E[:tw, kst:ken])
                            wb = sm.tile([128, 128], BF16, tag="wb")
                            src2 = eb[:]
                            v2 = src2.ap
                            v2.clear()
                            v2.extend([[257, tw], [1, 127]])
                            src2.offset = 65 - cA
                            nc.sync.dma_start(wb[:tw, 0:127], src2)
                            if q0 < 64:
                                # zero cells with k = q0+p+c+1-64 < 0
                                nc.gpsimd.affine_select(
                                    out=wb[:tw, 0:127], in_=wb[:tw, 0:127],
                                    compare_op=ALU.is_ge, fill=0.0,
                                    base=q0 - 63, channel_multiplier=1,
                                    pattern=[[1, 127]])
                            if q0 + tw + 63 > S - 1:
                                # zero cells with k = q0+p+c+1-64 > S-1
                                nc.gpsimd.affine_select(
                                    out=wb[:tw, 0:127], in_=wb[:tw, 0:127],
                                    compare_op=ALU.is_ge, fill=0.0,
                                    base=S + 62 - q0, channel_multiplier=-1,
                                    pattern=[[-1, 127]])
                            wbT = sm.tile([128, 128], BF16, tag="wbT")
                            nc.sync.dma_start_transpose(wbT[:, :], wb[:, :])

                        # E^T blocks via XBAR transpose into the group tile
                        gidx = qt // 4          # 3 groups of 4 q-tiles
                        qoff = (qt % 4) * 128   # column offset within group
                        if qt % 4 == 0:
                            ET4 = hdp.tile([128, NQT, 512], BF16, tag="ET4")
                            wbTs = []
                            lrTs = []
                        if level >= 13:
                            nc.sync.dma_start_transpose(
                                ET4[:, :, qoff:qoff + 128], E[:, 0:SP])
                        if level >= 33:
                            wbTs.append(wbT)
                            lrTs.append(lrT)

                        # at group end: out^T = w @ V + rel-v, one PSUM group
                        if level >= 14 and (qt % 4 == 3 or qt == NQT - 1):
                            g0 = gidx * 512
                            gw = min(S, g0 + 512) - g0
                            psO = psOp.tile([HD, 512], F32, tag="psO")
                            for kt2 in range(NQT):
                                nc.tensor.matmul(psO[:, :gw], v_sb[:, kt2, :],
                                                 ET4[:, kt2, :gw],
                                                 start=(kt2 == 0),
                                                 stop=(kt2 == NQT - 1 and level < 33))
                            if level >= 33:
                                for j in range(len(wbTs)):
                                    tj = QTS[gidx * 4 + j]
                                    o0 = j * 128
                                    nc.tensor.matmul(psO[:, o0:o0 + tj], pvi_sb[:, :],
                                                     wbTs[j][0:127, :tj],
                                                     start=False, stop=False)
                                    nc.tensor.matmul(psO[:, o0:o0 + tj], pvt_sb[:, :],
                                                     lrTs[j][:, :tj],
                                                     start=False,
                                                     stop=(j == len(wbTs) - 1))
                            nc.scalar.copy(
                                hid_sb[64 * (n % 2):64 * (n % 2) + HD, n // 2, g0:g0 + gw],
                                psO[:, :gw])
                        scnt += 1

                # fc projection for this batch
                for qt in range(NQT):
                    tw = QTS[qt]
                    q0 = qt * 128
                    for oc in range(2):
                        psF = psAp.tile([128, 512], F32, tag="psA")
                        for c in range(8):
                            nc.tensor.matmul(psF[:tw, :], hid_sb[:, c, q0:q0 + tw],
                                             wfc_sb[:, c, oc * 512:(oc + 1) * 512],
                                             start=(c == 0), stop=False)
                        nc.tensor.matmul(psF[:tw, :], ones[:, :tw],
                                         bfr_sb[:, oc * 512:(oc + 1) * 512],
                                         start=False, stop=True)
                        ob = sm.tile([128, 512], BF16, tag="ob")
                        nc.scalar.copy(ob[:tw, :], psF[:tw, :])
                        nc.sync.dma_start(
                            out[q0:q0 + tw, oc * 512:(oc + 1) * 512],
                            ob[:tw, :])

    nc.compile()
    return nc


def kernel(query, key, value, Wq, bq, Wk, bk, Wv, bv, pe_k, pe_v, W_fc, b_fc):
    from concourse.bass_utils import run_bass_kernel_spmd

    query = np.asarray(query, np.float32)
    key = np.asarray(key, np.float32)
    value = np.asarray(value, np.float32)

    qTb = np.ascontiguousarray(query.transpose(0, 2, 1)).astype(BF)
    kTb = np.ascontiguousarray(key.transpose(0, 2, 1)).astype(BF)
    vTb = np.ascontiguousarray(value.transpose(0, 2, 1)).astype(BF)
    in_map = {
        "wq": (np.asarray(Wq, np.float32) / 8.0).astype(BF),
        "wk": np.asarray(Wk, BF),
        "wv": np.asarray(Wv, BF),
        "bqT": np.ascontiguousarray(np.asarray(bq, np.float32).T / 8.0),
        "bkT": np.ascontiguousarray(np.asarray(bk, np.float32).T),
        "bvT": np.ascontiguousarray(np.asarray(bv, np.float32).T),
        "pkT": np.ascontiguousarray(np.asarray(pe_k, np.float32).T).astype(BF),
        "pvi": np.asarray(pe_v[1:128], BF),
        "pvt": np.asarray(pe_v[[0, 128]], BF),
        "wfc": np.asarray(W_fc, BF),
        "bfr": np.asarray(b_fc, BF).reshape(1, HID),
    }

    if "nc" not in _CACHE:
        _CACHE["nc"] = _build()
    nc = _CACHE["nc"]

    in_maps = [
        {**in_map, "qT": qTb[b], "kT": kTb[b], "vT": vTb[b]} for b in range(B)
    ]
    res = run_bass_kernel_spmd(nc, in_maps, [0, 1])
    o = np.stack([np.asarray(res.results[c]["o"], np.float32) for c in range(B)])
    return o.reshape(B, S, HID)
